# revision 15
# baseline (speedup 1.0000x reference)
"""Trainium2 Bass kernel for nn_BinaryLinear (binarized 4-layer MLP + BatchNorm).

Reference computation (fp32, jax):
    h = x.reshape(-1, 3072)
    h = relu(h @ sign(W1).T); h = BN(h, g1, b1)   # BN over full 8192 batch
    h = relu(h @ sign(W2).T); h = BN(h, g2, b2)
    h = relu(h @ sign(W3).T); h = BN(h, g3, b3)
    out = h @ sign(W4).T                          # [8192, 10]

Strategy (8 NeuronCores, data-parallel over batch):
  - Host: binarize weights to bf16 (+-1 exact), transpose everything into
    feature-major "lhsT"/"rhs" layouts, shard x over cores (1024 rows each).
  - Device (SPMD identical program): activations live feature-major
    [feature_part, batch_free] in SBUF; each layer is a K-tiled bf16 matmul
    accumulating in PSUM, relu via ScalarE (with free per-partition sum
    accum_out for BN stats), sum(h^2) via VectorE tensor_tensor_reduce.
  - BatchNorm over the full batch: AllGather the per-core (sum, sumsq) stats
    (one [128,16] f32 tile per layer), reduce locally, apply a*h+c per
    feature via VectorE tensor_scalar.
"""
import os
import sys

for _p in ("/opt/trn_rl_repo",):
    if os.path.isdir(_p) and _p not in sys.path:
        sys.path.insert(0, _p)

import numpy as np
import ml_dtypes

from concourse import bacc, tile, mybir
from concourse import bass_utils

NCORES = 8
B = 8192
BL = B // NCORES            # 1024 rows per core
KIN = 3072
KT_IN = KIN // 128          # 24 k-tiles for layer 1
HID = 1024
JT = HID // 128             # 8 feature tiles
CLS = 10
CLSP = 16                   # padded classes
EPS = 1e-5
BF16 = mybir.dt.bfloat16
F32 = mybir.dt.float32
ADD = mybir.AluOpType.add
SUB = mybir.AluOpType.subtract
MUL = mybir.AluOpType.mult
RELU = mybir.ActivationFunctionType.Relu

_CACHE = {}
NO_ACCUM = os.environ.get("NO_ACCUM", "0") == "1"
NO_TTR = os.environ.get("NO_TTR", "0") == "1"


def _build(stage=99):
    nc = bacc.Bacc("TRN2", target_bir_lowering=False, debug=False, num_devices=NCORES)

    xt_d = nc.dram_tensor("xt", [KIN, BL], BF16, kind="ExternalInput")
    w1_d = nc.dram_tensor("w1t", [KIN, HID], BF16, kind="ExternalInput")
    w2_d = nc.dram_tensor("w2t", [HID, HID], BF16, kind="ExternalInput")
    w3_d = nc.dram_tensor("w3t", [HID, HID], BF16, kind="ExternalInput")
    w4_d = nc.dram_tensor("w4t", [HID, CLSP], BF16, kind="ExternalInput")
    bnp_d = nc.dram_tensor("bnp", [128, 6 * JT], F32, kind="ExternalInput")
    out_d = nc.dram_tensor("out", [CLSP, BL], F32, kind="ExternalOutput")

    with tile.TileContext(nc) as tc:
        with (
            tc.tile_pool(name="weights", bufs=1) as wpool,
            tc.tile_pool(name="acts", bufs=1) as apool,
            tc.tile_pool(name="scratch", bufs=2) as scrpool,
            tc.tile_pool(name="stats", bufs=2) as spool,
            tc.tile_pool(name="psum", bufs=3, space="PSUM") as pspool,
            tc.tile_pool(name="psum4", bufs=1, space="PSUM") as ps4pool,
            tc.tile_pool(name="dram", bufs=2, space="DRAM") as dpool,
        ):
            XT = wpool.tile([128, KT_IN, BL], BF16, tag="XT")
            W1 = wpool.tile([128, KT_IN, HID], BF16, tag="W1")
            W2 = wpool.tile([128, JT, HID], BF16, tag="W2")
            W3 = wpool.tile([128, JT, HID], BF16, tag="W3")
            W4 = wpool.tile([128, JT, CLSP], BF16, tag="W4")
            BNP = wpool.tile([128, 6 * JT], F32, tag="BNP")
            HRAW = apool.tile([128, JT, BL], BF16, tag="HRAW")
            H = apool.tile([128, JT, BL], BF16, tag="H")

            # Warmup collective: absorbs the ~11us ncfw wake latency off the
            # critical path. Output anchored into an unused out_d row so DCE
            # keeps it.
            WUS = spool.tile([128, 1], F32, tag="WUS")
            nc.vector.memset(WUS[:], 0.0)
            wu_in = dpool.tile([128, 1], F32, tag="wu_in")
            wu_out = dpool.tile([NCORES * 128, 1], F32, tag="wu_out")
            nc.sync.dma_start(wu_in[:], WUS[:])
            nc.gpsimd.collective_compute(
                "AllGather",
                mybir.AluOpType.bypass,
                replica_groups=[list(range(NCORES))],
                ins=[wu_in.opt()],
                outs=[wu_out.opt()],
            )
            nc.sync.dma_start(out_d[CLSP - 1 : CLSP, 0:1], wu_out[0:1, :])

            # Input DMAs: contiguous per-k-tile transfers (fast descriptor
            # gen); XT triggers on Sync (HWDGE), W1 triggers on GpSimd so the
            # two trigger streams issue in parallel.
            nc.sync.dma_start(BNP[:], bnp_d[:])
            for k in range(KT_IN):
                nc.sync.dma_start(XT[:, k, :], xt_d[k * 128 : (k + 1) * 128, :])
                nc.gpsimd.dma_start(W1[:, k, :], w1_d[k * 128 : (k + 1) * 128, :])

            nhalves = [(s, min(512, BL - s)) for s in range(0, BL, 512)]

            def mlp_layer(kt, rhs, W, S):
                """matmul(K=kt*128) + relu + per-core BN stats into S[128,16]."""
                for jt in range(JT):
                    ps = pspool.tile([128, BL], F32, tag="ps")
                    # k-outer, halves inner: consecutive matmuls share the
                    # same stationary weights (one LDWEIGHTS per 2 matmuls)
                    for k in range(kt):
                        for s, w in nhalves:
                            nc.tensor.matmul(
                                ps[:, s : s + w],
                                W[:, k, jt * 128 : (jt + 1) * 128],
                                rhs[:, k, s : s + w],
                                start=(k == 0),
                                stop=(k == kt - 1),
                            )
                    # relu: PSUM f32 -> SBUF bf16; accum_out = sum over batch
                    if NO_ACCUM:
                        nc.scalar.activation(HRAW[:, jt, :], ps[:], RELU)
                        nc.vector.memset(S[:, jt : jt + 1], 0.0)
                    else:
                        nc.scalar.activation(
                            HRAW[:, jt, :], ps[:], RELU,
                            accum_out=S[:, jt : jt + 1],
                        )
                    # sum of squares over batch (ScalarE Square + accum)
                    if NO_TTR:
                        nc.vector.memset(S[:, JT + jt : JT + jt + 1], 0.0)
                    else:
                        scr = scrpool.tile([128, BL], BF16, tag="scr")
                        nc.scalar.activation(
                            scr[:],
                            HRAW[:, jt, :],
                            mybir.ActivationFunctionType.Square,
                            accum_out=S[:, JT + jt : JT + jt + 1],
                        )

            def bn_sync_apply(li, S):
                """AllGather per-core stats, compute a/c, H = a*HRAW + c."""
                cc_in = dpool.tile([128, 2 * JT], F32, tag="cc_in")
                cc_out = dpool.tile([NCORES * 128, 2 * JT], F32, tag="cc_out")
                nc.sync.dma_start(cc_in[:], S[:])
                nc.gpsimd.collective_compute(
                    "AllGather",
                    mybir.AluOpType.bypass,
                    replica_groups=[list(range(NCORES))],
                    ins=[cc_in.opt()],
                    outs=[cc_out.opt()],
                )
                GAT = spool.tile([128, NCORES, 2 * JT], F32, tag="GAT")
                nc.sync.dma_start(
                    GAT[:], cc_out.opt().rearrange("(c p) s -> p c s", p=128)
                )
                T4 = spool.tile([128, 4, 2 * JT], F32, tag="T4")
                nc.vector.tensor_tensor(T4[:], GAT[:, 0:4, :], GAT[:, 4:8, :], ADD)
                T2 = spool.tile([128, 2, 2 * JT], F32, tag="T2")
                nc.vector.tensor_tensor(T2[:], T4[:, 0:2, :], T4[:, 2:4, :], ADD)
                SS = spool.tile([128, 2 * JT], F32, tag="SS")
                nc.vector.tensor_tensor(SS[:], T2[:, 0, :], T2[:, 1, :], ADD)

                MEAN = spool.tile([128, JT], F32, tag="MEAN")
                nc.vector.tensor_scalar_mul(MEAN[:], SS[:, 0:JT], 1.0 / B)
                # E[h^2] + eps
                VPE = spool.tile([128, JT], F32, tag="VPE")
                nc.vector.tensor_scalar(
                    VPE[:], SS[:, JT : 2 * JT], 1.0 / B, EPS, MUL, ADD
                )
                MSQ = spool.tile([128, JT], F32, tag="MSQ")
                nc.vector.tensor_tensor(MSQ[:], MEAN[:], MEAN[:], MUL)
                VAR = spool.tile([128, JT], F32, tag="VAR")
                nc.vector.tensor_tensor(VAR[:], VPE[:], MSQ[:], SUB)  # var+eps
                RINV = spool.tile([128, JT], F32, tag="RINV")
                nc.vector.reciprocal(RINV[:], VAR[:])
                RSTD = spool.tile([128, JT], F32, tag="RSTD")
                nc.scalar.sqrt(RSTD[:], RINV[:])
                A = spool.tile([128, JT], F32, tag="A")
                nc.vector.tensor_tensor(
                    A[:], RSTD[:], BNP[:, (2 * li) * JT : (2 * li + 1) * JT], MUL
                )
                AM = spool.tile([128, JT], F32, tag="AM")
                nc.vector.tensor_tensor(AM[:], A[:], MEAN[:], MUL)
                C = spool.tile([128, JT], F32, tag="C")
                nc.vector.tensor_tensor(
                    C[:], BNP[:, (2 * li + 1) * JT : (2 * li + 2) * JT], AM[:], SUB
                )
                for jt in range(JT):
                    nc.vector.tensor_scalar(
                        H[:, jt, :],
                        HRAW[:, jt, :],
                        A[:, jt : jt + 1],
                        C[:, jt : jt + 1],
                        MUL,
                        ADD,
                    )

            # ---- layer 1 ----
            if stage >= 1:
                S1 = spool.tile([128, 2 * JT], F32, tag="S")
                mlp_layer(KT_IN, XT, W1, S1)
            # prefetch W2 during L1 compute (program order after L1 matmuls)
            for k in range(JT):
                nc.sync.dma_start(W2[:, k, :], w2_d[k * 128 : (k + 1) * 128, :])
            if stage >= 2:
                bn_sync_apply(0, S1)

            if stage >= 3:
                # ---- layer 2 ----
                S2 = spool.tile([128, 2 * JT], F32, tag="S")
                mlp_layer(JT, H, W2, S2)
            for k in range(JT):
                nc.sync.dma_start(W3[:, k, :], w3_d[k * 128 : (k + 1) * 128, :])
            if stage >= 3:
                bn_sync_apply(1, S2)

            if stage >= 4:
                # ---- layer 3 ----
                S3 = spool.tile([128, 2 * JT], F32, tag="S")
                mlp_layer(JT, H, W3, S3)
            nc.sync.dma_start(W4[:], w4_d.ap().rearrange("(k p) c -> p k c", p=128))
            if stage >= 4:
                bn_sync_apply(2, S3)

            OUTS = spool.tile([CLSP, BL], F32, tag="OUTS")
            if stage >= 5:
                # ---- layer 4 (no relu/BN) ----
                ps4 = ps4pool.tile([CLSP, BL], F32, tag="ps4")
                for s, w in nhalves:
                    for k in range(JT):
                        nc.tensor.matmul(
                            ps4[:, s : s + w],
                            W4[:, k, :],
                            H[:, k, s : s + w],
                            start=(k == 0),
                            stop=(k == JT - 1),
                        )
                nc.scalar.copy(OUTS[:], ps4[:])
            else:
                nc.vector.memset(OUTS[:], 0.0)
            nc.sync.dma_start(out_d[:], OUTS[:])

    nc.compile()
    return nc


def _get_nc():
    if "nc" not in _CACHE:
        _CACHE["nc"] = _build()
    return _CACHE["nc"]


def _prep_inputs(x, W1, W2, W3, W4, g1, b1, g2, b2, g3, b3):
    x2 = np.asarray(x, dtype=np.float32).reshape(B, KIN)
    xt = np.ascontiguousarray(x2.T).astype(ml_dtypes.bfloat16)  # [3072, 8192]

    def bin_t(w, pad=None):
        wb = np.where(np.asarray(w, dtype=np.float32) >= 0, 1.0, -1.0)
        wt = np.ascontiguousarray(wb.T).astype(ml_dtypes.bfloat16)  # [in, out]
        if pad is not None and wt.shape[1] < pad:
            wt = np.concatenate(
                [wt, np.zeros((wt.shape[0], pad - wt.shape[1]), wt.dtype)], axis=1
            )
        return wt

    w1t = bin_t(W1)          # [3072, 1024]
    w2t = bin_t(W2)          # [1024, 1024]
    w3t = bin_t(W3)
    w4t = bin_t(W4, pad=CLSP)  # [1024, 16]

    bnp = np.zeros((128, 6 * JT), dtype=np.float32)
    for l, p in enumerate([g1, b1, g2, b2, g3, b3]):
        pa = np.asarray(p, dtype=np.float32)
        for jt in range(JT):
            bnp[:, l * JT + jt] = pa[jt * 128 : (jt + 1) * 128]

    shared = {"w1t": w1t, "w2t": w2t, "w3t": w3t, "w4t": w4t, "bnp": bnp}
    in_maps = []
    for c in range(NCORES):
        m = dict(shared)
        m["xt"] = np.ascontiguousarray(xt[:, c * BL : (c + 1) * BL])
        in_maps.append(m)
    return in_maps


def _run(inputs, trace=False):
    nc = _get_nc()
    in_maps = _prep_inputs(**inputs)
    res = bass_utils.run_bass_kernel_spmd(
        nc, in_maps, core_ids=list(range(NCORES)), trace=trace
    )
    out = np.empty((B, CLS), dtype=np.float32)
    for c in range(NCORES):
        out[c * BL : (c + 1) * BL, :] = res.results[c]["out"][:CLS, :].T
    return out, res


def kernel(**inputs):
    out, _ = _run(inputs, trace=False)
    return out


# revision 16
# speedup vs baseline: 1.0913x; 1.0913x over previous
"""Trainium2 Bass kernel for nn_BinaryLinear (binarized 4-layer MLP + BatchNorm).

Reference computation (fp32, jax):
    h = x.reshape(-1, 3072)
    h = relu(h @ sign(W1).T); h = BN(h, g1, b1)   # BN over full 8192 batch
    h = relu(h @ sign(W2).T); h = BN(h, g2, b2)
    h = relu(h @ sign(W3).T); h = BN(h, g3, b3)
    out = h @ sign(W4).T                          # [8192, 10]

Strategy (8 NeuronCores, data-parallel over batch):
  - Host: binarize weights to bf16 (+-1 exact), transpose everything into
    feature-major "lhsT"/"rhs" layouts, shard x over cores (1024 rows each).
  - Device (SPMD identical program): activations live feature-major
    [feature_part, batch_free] in SBUF; each layer is a K-tiled bf16 matmul
    accumulating in PSUM, relu via ScalarE (with free per-partition sum
    accum_out for BN stats), sum(h^2) via VectorE tensor_tensor_reduce.
  - BatchNorm over the full batch: AllGather the per-core (sum, sumsq) stats
    (one [128,16] f32 tile per layer), reduce locally, apply a*h+c per
    feature via VectorE tensor_scalar.
"""
import os
import sys

for _p in ("/opt/trn_rl_repo",):
    if os.path.isdir(_p) and _p not in sys.path:
        sys.path.insert(0, _p)

import numpy as np
import ml_dtypes

from concourse import bacc, tile, mybir
from concourse import bass_utils

NCORES = 8
B = 8192
BL = B // NCORES            # 1024 rows per core
KIN = 3072
KT_IN = KIN // 128          # 24 k-tiles for layer 1
HID = 1024
JT = HID // 128             # 8 feature tiles
CLS = 10
CLSP = 16                   # padded classes
EPS = 1e-5
BF16 = mybir.dt.bfloat16
F32 = mybir.dt.float32
ADD = mybir.AluOpType.add
SUB = mybir.AluOpType.subtract
MUL = mybir.AluOpType.mult
RELU = mybir.ActivationFunctionType.Relu

_CACHE = {}
NO_ACCUM = os.environ.get("NO_ACCUM", "0") == "1"
NO_TTR = os.environ.get("NO_TTR", "0") == "1"


def _build(stage=99):
    nc = bacc.Bacc("TRN2", target_bir_lowering=False, debug=False, num_devices=NCORES)

    xt_d = nc.dram_tensor("xt", [KIN, BL], BF16, kind="ExternalInput")
    w1_d = nc.dram_tensor("w1t", [KIN, HID], BF16, kind="ExternalInput")
    w2_d = nc.dram_tensor("w2t", [HID, HID], BF16, kind="ExternalInput")
    w3_d = nc.dram_tensor("w3t", [HID, HID], BF16, kind="ExternalInput")
    w4_d = nc.dram_tensor("w4t", [HID, CLSP], BF16, kind="ExternalInput")
    bnp_d = nc.dram_tensor("bnp", [128, 6 * JT], F32, kind="ExternalInput")
    out_d = nc.dram_tensor("out", [CLSP, BL], F32, kind="ExternalOutput")

    with tile.TileContext(nc) as tc:
        with (
            tc.tile_pool(name="weights", bufs=1) as wpool,
            tc.tile_pool(name="acts", bufs=1) as apool,
            tc.tile_pool(name="scratch", bufs=2) as scrpool,
            tc.tile_pool(name="stats", bufs=2) as spool,
            tc.tile_pool(name="psum", bufs=3, space="PSUM") as pspool,
            tc.tile_pool(name="psum4", bufs=1, space="PSUM") as ps4pool,
            tc.tile_pool(name="dram", bufs=2, space="DRAM") as dpool,
        ):
            XT = wpool.tile([128, KT_IN, BL], BF16, tag="XT")
            W1 = wpool.tile([128, KT_IN, HID], BF16, tag="W1")
            W2 = wpool.tile([128, JT, HID], BF16, tag="W2")
            W3 = wpool.tile([128, JT, HID], BF16, tag="W3")
            W4 = wpool.tile([128, JT, CLSP], BF16, tag="W4")
            BNP = wpool.tile([128, 6 * JT], F32, tag="BNP")
            HRAW = apool.tile([128, JT, BL], BF16, tag="HRAW")
            H = apool.tile([128, JT, BL], BF16, tag="H")

            # Warmup collective: absorbs the ~11us ncfw wake latency off the
            # critical path. Input is an unwritten scratch buffer (contents
            # irrelevant) so the trigger fires immediately; output anchored
            # into an unused out_d row so DCE keeps it.
            wu_in = dpool.tile([128, 1], F32, tag="wu_in")
            wu_out = dpool.tile([NCORES * 128, 1], F32, tag="wu_out")
            nc.gpsimd.collective_compute(
                "AllGather",
                mybir.AluOpType.bypass,
                replica_groups=[list(range(NCORES))],
                ins=[wu_in.opt()],
                outs=[wu_out.opt()],
            )
            nc.sync.dma_start(out_d[CLSP - 1 : CLSP, 0:1], wu_out[0:1, :])

            # Input DMAs: contiguous per-k-tile transfers. XT triggers on the
            # Sync HWDGE ring, W1 on the Scalar HWDGE ring — two parallel
            # FIFO trigger streams.
            nc.sync.dma_start(BNP[:], bnp_d[:])
            for k in range(KT_IN):
                nc.sync.dma_start(XT[:, k, :], xt_d[k * 128 : (k + 1) * 128, :])
                nc.scalar.dma_start(W1[:, k, :], w1_d[k * 128 : (k + 1) * 128, :])

            nhalves = [(s, min(512, BL - s)) for s in range(0, BL, 512)]

            def mlp_layer(kt, rhs, W, S):
                """matmul(K=kt*128) + relu + per-core BN stats into S[128,16]."""
                for jt in range(JT):
                    ps = pspool.tile([128, BL], F32, tag="ps")
                    # k-outer, halves inner: consecutive matmuls share the
                    # same stationary weights (one LDWEIGHTS per 2 matmuls)
                    for k in range(kt):
                        for s, w in nhalves:
                            nc.tensor.matmul(
                                ps[:, s : s + w],
                                W[:, k, jt * 128 : (jt + 1) * 128],
                                rhs[:, k, s : s + w],
                                start=(k == 0),
                                stop=(k == kt - 1),
                            )
                    # relu: PSUM f32 -> SBUF bf16; accum_out = sum over batch
                    if NO_ACCUM:
                        nc.scalar.activation(HRAW[:, jt, :], ps[:], RELU)
                        nc.vector.memset(S[:, jt : jt + 1], 0.0)
                    else:
                        nc.scalar.activation(
                            HRAW[:, jt, :], ps[:], RELU,
                            accum_out=S[:, jt : jt + 1],
                        )
                    # sum of squares over batch (ScalarE Square + accum)
                    if NO_TTR:
                        nc.vector.memset(S[:, JT + jt : JT + jt + 1], 0.0)
                    else:
                        scr = scrpool.tile([128, BL], BF16, tag="scr")
                        nc.scalar.activation(
                            scr[:],
                            HRAW[:, jt, :],
                            mybir.ActivationFunctionType.Square,
                            accum_out=S[:, JT + jt : JT + jt + 1],
                        )

            def bn_sync_apply(li, S):
                """AllGather per-core stats, compute a/c, H = a*HRAW + c."""
                cc_in = dpool.tile([128, 2 * JT], F32, tag="cc_in")
                cc_out = dpool.tile([NCORES * 128, 2 * JT], F32, tag="cc_out")
                nc.sync.dma_start(cc_in[:], S[:])
                nc.gpsimd.collective_compute(
                    "AllGather",
                    mybir.AluOpType.bypass,
                    replica_groups=[list(range(NCORES))],
                    ins=[cc_in.opt()],
                    outs=[cc_out.opt()],
                )
                GAT = spool.tile([128, NCORES, 2 * JT], F32, tag="GAT")
                nc.sync.dma_start(
                    GAT[:], cc_out.opt().rearrange("(c p) s -> p c s", p=128)
                )
                T4 = spool.tile([128, 4, 2 * JT], F32, tag="T4")
                nc.vector.tensor_tensor(T4[:], GAT[:, 0:4, :], GAT[:, 4:8, :], ADD)
                T2 = spool.tile([128, 2, 2 * JT], F32, tag="T2")
                nc.vector.tensor_tensor(T2[:], T4[:, 0:2, :], T4[:, 2:4, :], ADD)
                SS = spool.tile([128, 2 * JT], F32, tag="SS")
                nc.vector.tensor_tensor(SS[:], T2[:, 0, :], T2[:, 1, :], ADD)

                MEAN = spool.tile([128, JT], F32, tag="MEAN")
                nc.vector.tensor_scalar_mul(MEAN[:], SS[:, 0:JT], 1.0 / B)
                # E[h^2] + eps
                VPE = spool.tile([128, JT], F32, tag="VPE")
                nc.vector.tensor_scalar(
                    VPE[:], SS[:, JT : 2 * JT], 1.0 / B, EPS, MUL, ADD
                )
                MSQ = spool.tile([128, JT], F32, tag="MSQ")
                nc.vector.tensor_tensor(MSQ[:], MEAN[:], MEAN[:], MUL)
                VAR = spool.tile([128, JT], F32, tag="VAR")
                nc.vector.tensor_tensor(VAR[:], VPE[:], MSQ[:], SUB)  # var+eps
                RINV = spool.tile([128, JT], F32, tag="RINV")
                nc.vector.reciprocal(RINV[:], VAR[:])
                RSTD = spool.tile([128, JT], F32, tag="RSTD")
                nc.scalar.sqrt(RSTD[:], RINV[:])
                A = spool.tile([128, JT], F32, tag="A")
                nc.vector.tensor_tensor(
                    A[:], RSTD[:], BNP[:, (2 * li) * JT : (2 * li + 1) * JT], MUL
                )
                AM = spool.tile([128, JT], F32, tag="AM")
                nc.vector.tensor_tensor(AM[:], A[:], MEAN[:], MUL)
                C = spool.tile([128, JT], F32, tag="C")
                nc.vector.tensor_tensor(
                    C[:], BNP[:, (2 * li + 1) * JT : (2 * li + 2) * JT], AM[:], SUB
                )
                for jt in range(JT):
                    nc.vector.tensor_scalar(
                        H[:, jt, :],
                        HRAW[:, jt, :],
                        A[:, jt : jt + 1],
                        C[:, jt : jt + 1],
                        MUL,
                        ADD,
                    )

            # ---- layer 1 ----
            if stage >= 1:
                S1 = spool.tile([128, 2 * JT], F32, tag="S")
                mlp_layer(KT_IN, XT, W1, S1)
            # prefetch W2 during L1 compute (program order after L1 matmuls)
            for k in range(JT):
                nc.sync.dma_start(W2[:, k, :], w2_d[k * 128 : (k + 1) * 128, :])
            if stage >= 2:
                bn_sync_apply(0, S1)

            if stage >= 3:
                # ---- layer 2 ----
                S2 = spool.tile([128, 2 * JT], F32, tag="S")
                mlp_layer(JT, H, W2, S2)
            for k in range(JT):
                nc.sync.dma_start(W3[:, k, :], w3_d[k * 128 : (k + 1) * 128, :])
            if stage >= 3:
                bn_sync_apply(1, S2)

            if stage >= 4:
                # ---- layer 3 ----
                S3 = spool.tile([128, 2 * JT], F32, tag="S")
                mlp_layer(JT, H, W3, S3)
            nc.sync.dma_start(W4[:], w4_d.ap().rearrange("(k p) c -> p k c", p=128))
            if stage >= 4:
                bn_sync_apply(2, S3)

            OUTS = spool.tile([CLSP, BL], F32, tag="OUTS")
            if stage >= 5:
                # ---- layer 4 (no relu/BN) ----
                ps4 = ps4pool.tile([CLSP, BL], F32, tag="ps4")
                for s, w in nhalves:
                    for k in range(JT):
                        nc.tensor.matmul(
                            ps4[:, s : s + w],
                            W4[:, k, :],
                            H[:, k, s : s + w],
                            start=(k == 0),
                            stop=(k == JT - 1),
                        )
                nc.scalar.copy(OUTS[:], ps4[:])
            else:
                nc.vector.memset(OUTS[:], 0.0)
            nc.sync.dma_start(out_d[:], OUTS[:])

    nc.compile()
    return nc


def _get_nc():
    if "nc" not in _CACHE:
        _CACHE["nc"] = _build()
    return _CACHE["nc"]


def _prep_inputs(x, W1, W2, W3, W4, g1, b1, g2, b2, g3, b3):
    x2 = np.asarray(x, dtype=np.float32).reshape(B, KIN)
    xt = np.ascontiguousarray(x2.T).astype(ml_dtypes.bfloat16)  # [3072, 8192]

    def bin_t(w, pad=None):
        wb = np.where(np.asarray(w, dtype=np.float32) >= 0, 1.0, -1.0)
        wt = np.ascontiguousarray(wb.T).astype(ml_dtypes.bfloat16)  # [in, out]
        if pad is not None and wt.shape[1] < pad:
            wt = np.concatenate(
                [wt, np.zeros((wt.shape[0], pad - wt.shape[1]), wt.dtype)], axis=1
            )
        return wt

    w1t = bin_t(W1)          # [3072, 1024]
    w2t = bin_t(W2)          # [1024, 1024]
    w3t = bin_t(W3)
    w4t = bin_t(W4, pad=CLSP)  # [1024, 16]

    bnp = np.zeros((128, 6 * JT), dtype=np.float32)
    for l, p in enumerate([g1, b1, g2, b2, g3, b3]):
        pa = np.asarray(p, dtype=np.float32)
        for jt in range(JT):
            bnp[:, l * JT + jt] = pa[jt * 128 : (jt + 1) * 128]

    shared = {"w1t": w1t, "w2t": w2t, "w3t": w3t, "w4t": w4t, "bnp": bnp}
    in_maps = []
    for c in range(NCORES):
        m = dict(shared)
        m["xt"] = np.ascontiguousarray(xt[:, c * BL : (c + 1) * BL])
        in_maps.append(m)
    return in_maps


def _run(inputs, trace=False):
    nc = _get_nc()
    in_maps = _prep_inputs(**inputs)
    res = bass_utils.run_bass_kernel_spmd(
        nc, in_maps, core_ids=list(range(NCORES)), trace=trace
    )
    out = np.empty((B, CLS), dtype=np.float32)
    for c in range(NCORES):
        out[c * BL : (c + 1) * BL, :] = res.results[c]["out"][:CLS, :].T
    return out, res


def kernel(**inputs):
    out, _ = _run(inputs, trace=False)
    return out
